# revision 102
# baseline (speedup 1.0000x reference)
"""Multi-head attention (RoPE, causal) Trainium2 Bass kernel, v4.

Problem: nn_MultiHeadAttention_62431644615193
  x: [2, 2048, 1024] f32, mask: causal tril, w_qkv: [1024, 3072], w_out: [1024, 1024]

Sharding: 8 cores = batch(2) x head-groups(4 heads each). Each core emits a
bf16 partial [2048, 1024] (its heads through w_out rows); host sums 4
partials per batch in f32.

v4 design (157.9us -> 126.7us cost-model):
  - pv matmuls FLIPPED: out[128 queries, hd+1] per (query-slice o, head)
    accumulation region (lhsT = phat 128x128 slice, rhs = va[keys, 65]).
    M=65 -> 128 saves ~14us PE; the softmax denominator (ones column of
    va) lands at psum col 64 = per-partition, so normalization is one
    per-partition DVE mul (rec broadcast along hd). Kills v2's gpsimd
    partition broadcasts, den copies, and un-staging.
  - PSUM START IS BANK-GRANULAR: start=True marks the whole 2KB bank
    pending-zero (lazily applied at each byte's next matmul touch), so
    with 4 accumulation regions packed per pv bank exactly ONE matmul per
    bank per chunk carries start=True; every other region gets its fresh
    start via the bank's pending-zero at first touch, then accumulates.
  - STAGGER: region o's first pv matmul is deferred to unit o (catch-up
    burst) so a new chunk's writes never collide with the previous
    chunk's pending normalize reads (pvA: o=0,1 / pvB: o=2,3, bufs=1).
  - ao [q, f] -> [f, q] via DMA XBAR transpose (14ns/16x128 tile, off all
    compute engines); last chunk hp1 uses PE ident-transposes instead
    (shorter tail chain). outproj consumes ao tiles unchanged.
  - GPSIMD cannot access PSUM: all psum drains are DVE/ACT. ACT takes
    low-bass_priority copies (exp always wins); DVE reads that release
    aux psum (qk_raw, rotsin, va) are high-priority.
  - Cross-chunk qk+exp PREFETCH (PFQ) pulls exp work into ACT-slack
    windows of earlier chunks (chunk 3's first 5 units run inside 2,1).
  - Filler schedule: outproj(i) deferred 1-2 chunks (aopool bufs=8);
    quarter-3 k-proj/v-proj deferred into chunk 3; outproj tail split-DMA
    on alternating SP/ACT queues.
  - PSUM: qk 2x[128,2,512] (4 banks) + pvA/pvB [128,2,2,65] (2) + aux
    2x[128,512] (2) = 8 banks exactly.
"""

import math
import os

import numpy as np
import ml_dtypes

DEBUG_TAPS = bool(os.environ.get("KERNEL_DEBUG_TAPS"))

import concourse.bass as bass
import concourse.tile as tile
from concourse import bacc
import concourse.mybir as mybir
from concourse.bass_utils import run_bass_kernel_spmd

B, S, D = 2, 2048, 1024
H = 16
HD = D // H          # 64
HPC = H // 4         # 4 heads per core
ROPE_BASE = 10000.0

F32 = mybir.dt.float32
F32R = mybir.dt.float32r
BF16 = mybir.dt.bfloat16
AF = mybir.ActivationFunctionType

NT = S // 128        # 16 t-blocks
NI = 4               # 512-wide s-chunks

SECTIONS: list = []  # (start_instruction_id, label) in emit order


def build_nc(causal: bool = True):
    nc = bacc.Bacc("TRN2", target_bir_lowering=False, debug=False, num_devices=8)
    SECTIONS.clear()

    def mark(label):
        SECTIONS.append((nc.next_id(), label))

    xT = nc.dram_tensor("xT", [D, S], BF16, kind="ExternalInput")
    w_qk = nc.dram_tensor("w_qk", [D, 8 * HD], BF16, kind="ExternalInput")
    w_v = nc.dram_tensor("w_v", [D, 4 * HD], BF16, kind="ExternalInput")
    w_out = nc.dram_tensor("w_out", [4 * HD, D], BF16, kind="ExternalInput")
    cos2 = nc.dram_tensor("cos2", [128, S], BF16, kind="ExternalInput")
    sin2 = nc.dram_tensor("sin2", [128, S], BF16, kind="ExternalInput")
    rotP = nc.dram_tensor("rotP", [128, 128], BF16, kind="ExternalInput")
    ident = nc.dram_tensor("ident", [128, 128], BF16, kind="ExternalInput")
    mask01 = nc.dram_tensor("mask01", [128, 2 * 128], BF16, kind="ExternalInput")
    outp = nc.dram_tensor("outp", [S, D], BF16, kind="ExternalOutput")
    if DEBUG_TAPS:
        d_qkT = nc.dram_tensor("d_qkT", [4, 128, S], BF16, kind="ExternalOutput")
        d_ao = nc.dram_tensor("d_ao", [NI, 2, 128, 512], BF16, kind="ExternalOutput")

    with tile.TileContext(nc) as tc:
        with (
            tc.tile_pool(name="const", bufs=1) as cpool,
            tc.tile_pool(name="qkT", bufs=1) as qkTpool,
            tc.tile_pool(name="va", bufs=1) as vapool,
            tc.tile_pool(name="xq", bufs=2) as xqpool,
            tc.tile_pool(name="qkraw", bufs=2) as qkrawpool,
            tc.tile_pool(name="ropescratch", bufs=2) as rspool,
            tc.tile_pool(name="phat", bufs=15) as phatpool,
            tc.tile_pool(name="norm", bufs=4) as normpool,
            tc.tile_pool(name="aoq", bufs=10) as aoqpool,
            tc.tile_pool(name="attn_out", bufs=8) as aopool,
            tc.tile_pool(name="outstage", bufs=4) as ostpool,
            tc.tile_pool(name="psqk", bufs=1, space="PSUM") as qkps,
            tc.tile_pool(name="pspv", bufs=1, space="PSUM") as pvps,
            tc.tile_pool(name="psaux", bufs=1, space="PSUM") as auxps,
        ):
            # ---------------- constants (consolidated tiles) ----------------
            w_qk_t = cpool.tile([128, 8, 8 * HD], BF16, name="wqkt", tag="wqkt")
            w_v_t = cpool.tile([128, 8, 4 * HD], BF16, name="wvt", tag="wvt")
            w_out_t = cpool.tile([128, 2, D], BF16, name="woutt", tag="woutt")
            rotP_t = cpool.tile([128, 128], BF16)
            ident_t = cpool.tile([128, 128], BF16, name="ident_t", tag="ident_t")
            cos_t = cpool.tile([128, S], BF16, name="cos_t", tag="cos_t")
            sin_t = cpool.tile([128, S], BF16, name="sin_t", tag="sin_t")
            mask01_t = cpool.tile([128, 2, 128], BF16)

            # --- startup DMA plan: batched transfers (per-DMA overhead is
            # ~0.9us), consumption-ordered, alternating SP/ACT queues so the
            # serialized transfer pipe matches consumption order.
            def dd_slab(dram, a, b, cols=None):
                """dram rows [128a, 128b) as [128, b-a, cols] slab."""
                sl = dram[128 * a : 128 * b, :] if cols is None else dram[128 * a : 128 * b, cols]
                return sl.rearrange("(dd p) s -> p dd s", p=128)

            xq_tiles: dict[int, object] = {}

            def xq_first(q):
                xq = xqpool.tile([128, 8, 512], BF16, tag="xq", name=f"xq{q}")
                xq_tiles[q] = xq
                return xq

            def issue_xq(q, eng, split=(4,)):
                xq = xq_first(q)
                s_sl = slice(512 * q, 512 * q + 512)
                lo = 0
                for n in (*split, 8):
                    if n > lo:
                        eng.dma_start(xq[:, lo:n, :], dd_slab(xT, lo, n, s_sl))
                    lo = n

            va_t = vapool.tile([128, NT, 4, HD + 1], BF16)
            nc.gpsimd.memset(va_t[:, :, :, HD : HD + 1], 1.0)
            # prelude needs only w_qk cols 0:256 (q-pair0 + k-pair0); the
            # second half (cols 256:512, for pg(0,2)/pg(0,3) in chunk 0)
            # follows after the prelude-critical transfers
            cA = slice(0, 256)
            nc.sync.dma_start(w_qk_t[:, 0:2, cA], dd_slab(w_qk, 0, 2, cA))
            nc.scalar.dma_start(
                xq_first(0)[:, 0:2, :], dd_slab(xT, 0, 2, slice(0, 512))
            )
            nc.sync.dma_start(w_qk_t[:, 2:4, cA], dd_slab(w_qk, 2, 4, cA))
            nc.scalar.dma_start(
                xq_tiles[0][:, 2:4, :], dd_slab(xT, 2, 4, slice(0, 512))
            )
            nc.sync.dma_start(w_qk_t[:, 4:8, cA], dd_slab(w_qk, 4, 8, cA))
            nc.scalar.dma_start(
                xq_tiles[0][:, 4:8, :], dd_slab(xT, 4, 8, slice(0, 512))
            )
            nc.scalar.dma_start(rotP_t[:], rotP[:])
            nc.sync.dma_start(w_v_t[:, 0:4, :], dd_slab(w_v, 0, 4))
            nc.scalar.dma_start(cos_t[:, 0:512], cos2[:, 0:512])
            nc.sync.dma_start(w_v_t[:, 4:8, :], dd_slab(w_v, 4, 8))
            nc.scalar.dma_start(sin_t[:, 0:512], sin2[:, 0:512])
            cB = slice(256, 512)
            nc.sync.dma_start(w_qk_t[:, 0:8, cB], dd_slab(w_qk, 0, 8, cB))
            nc.sync.dma_start(
                mask01_t[:], mask01[:].rearrange("p (b s) -> p b s", b=2)
            )
            issue_xq(1, nc.sync)
            nc.scalar.dma_start(cos_t[:, 512:1024], cos2[:, 512:1024])
            nc.scalar.dma_start(sin_t[:, 512:1024], sin2[:, 512:1024])
            nc.sync.dma_start(cos_t[:, 1024:2048], cos2[:, 1024:2048])
            nc.sync.dma_start(sin_t[:, 1024:2048], sin2[:, 1024:2048])
            nc.sync.dma_start(
                w_out_t[:], w_out[:].rearrange("(kk p) s -> p kk s", p=128)
            )
            nc.scalar.dma_start(ident_t[:], ident[:])

            w_out_f = w_out_t[:].rearrange("p a b -> p (a b)")
            # rotated qT/kT (bf16): [q_h0;q_h1], [k_h0;k_h1], [q_h2;q_h3], [k_h2;k_h3]
            qkT = [qkTpool.tile([128, S], BF16, name=f"qkT{i}", tag=f"qkT{i}") for i in range(4)]

            # low-priority marker: the tile scheduler's heap picks lowest
            # bass_priority among READY instructions, so a large value makes
            # an op fill engine-idle gaps instead of delaying critical work
            _low_ctr = [1 << 20]

            def low_pri(bi):
                _low_ctr[0] += 1
                bi.ins.bass_priority = _low_ctr[0]
                return bi

            # ---------------- projection pieces ----------------
            def proj_group(q, mt):
                """qk-projection matmul group; returns qk_raw SBUF copy."""
                mark(f"pg{q}.{mt}")
                xq = xq_tiles[q]
                ps = auxps.tile([128, 512], F32, tag="aux", bufs=2, name="ps_g")
                for dd in range(8):
                    nc.tensor.matmul(
                        ps[:],
                        w_qk_t[:, dd, 128 * mt : 128 * mt + 128],
                        xq[:, dd, :],
                        start=(dd == 0),
                        stop=(dd == 7),
                    )
                qk_raw = qkrawpool.tile([128, 512], BF16, tag="qkraw", name="qk_raw")
                # PSUM reads are DVE/ACT-only (GPSIMD cannot access PSUM).
                # Quarter 1 runs while ACT has exp slack -> ACT low-pri;
                # quarter 0 (prelude; ACT SEQ is busy with DMA configs) and
                # later quarters go on DVE.
                if q == 1:
                    low_pri(nc.scalar.copy(qk_raw[:], ps[:]))
                else:
                    # high-pri: this read releases the aux psum buffer
                    with tc.high_priority():
                        nc.vector.tensor_copy(qk_raw[:], ps[:])
                return qk_raw

            def proj_rot(q, mt, qk_raw):
                """RoPE combine -> qkT[mt][:, quarter q] (bf16). rotate_half
                via a PE permutation matmul."""
                mark(f"rot{q}.{mt}")
                s_sl = slice(512 * q, 512 * q + 512)
                psr = auxps.tile([128, 512], F32, tag="aux", bufs=2, name="psr")
                nc.tensor.matmul(psr[:], rotP_t[:], qk_raw[:], start=True, stop=True)
                rotsin = rspool.tile([128, 512], BF16, tag="rs", name="rotsin")
                # high-pri: this read releases the aux psum buffer
                with tc.high_priority():
                    nc.vector.tensor_mul(rotsin[:], psr[:], sin_t[:, s_sl])
                qkcos = rspool.tile([128, 512], BF16, tag="qkcos", name="qkcos")
                nc.vector.tensor_mul(qkcos[:], qk_raw[:], cos_t[:, s_sl])
                nc.vector.tensor_add(qkT[mt][:, s_sl], qkcos[:], rotsin[:])

            def v_group(q, st):
                """v-projection for t-block j = 4q+st -> va (bf16)."""
                mark(f"vg{q}.{st}")
                j = 4 * q + st
                xq = xq_tiles[q]
                psv = auxps.tile([128, 4 * HD], F32, tag="aux", bufs=2, name="psv")
                for dd in range(8):
                    nc.tensor.matmul(
                        psv[:],
                        xq[:, dd, 128 * st : 128 * st + 128],
                        w_v_t[:, dd, :],
                        start=(dd == 0),
                        stop=(dd == 7),
                    )
                # high-pri: this read releases the aux psum buffer
                with tc.high_priority():
                    nc.vector.tensor_copy(
                        va_t[:, j, :, 0:HD], psv[:].rearrange("p (h c) -> p h c", h=4)
                    )

            # ---------------- outproj ----------------
            def outproj_tile(i, st, ao_pair, split_dma=False, use_qk_psum=False):
                mark(f"op{i}.{st}")
                ssl = slice(512 * i + 128 * st, 512 * i + 128 * st + 128)
                sloc = slice(128 * st, 128 * st + 128)
                ostage = ostpool.tile([128, 2, 512], BF16, tag="ost", name="ostage")
                # tail: the qk psum bufs are dead after the last unit; borrow
                # one per op tile so the final ops skip aux-psum recycling
                ps_pair = (
                    qkps.tile([128, 2, 512], F32, tag="qk", bufs=2, name="ps_ot")
                    if use_qk_psum else None
                )
                for n2 in range(2):
                    ps_o = (
                        ps_pair[:, n2, :] if ps_pair is not None
                        else auxps.tile([128, 512], F32, tag="aux", bufs=2, name="ps_o")[:]
                    )
                    for kk in range(2):
                        nc.tensor.matmul(
                            ps_o[:],
                            ao_pair[kk][:, sloc],
                            w_out_f[:, D * kk + 512 * n2 : D * kk + 512 * n2 + 512],
                            start=(kk == 0),
                            stop=(kk == 1),
                        )
                    # PSUM drains: ACT (low-pri, exp-slack windows) only for
                    # chunk-0 tiles and the tail (ACT idle there); chunk 1-3
                    # tiles fire inside ACT-saturated chunk 3 -> DVE
                    if n2 == 0 and (i == 0 or split_dma):
                        c = nc.scalar.copy(ostage[:, 0, :], ps_o[:])
                        if not split_dma:
                            low_pri(c)
                    else:
                        nc.vector.tensor_copy(ostage[:, n2, :], ps_o[:])
                    if split_dma:
                        # tail: per-half DMA on alternating SEQ queues so the
                        # configs overlap and the first transfer overlaps the
                        # second half's matmuls + copy
                        (nc.scalar if n2 == 0 else nc.sync).dma_start(
                            outp[ssl, 512 * n2 : 512 * n2 + 512], ostage[:, n2, :]
                        )
                if not split_dma:
                    nc.sync.dma_start(
                        outp[ssl, :], ostage[:].rearrange("p a b -> p (a b)")
                    )

            # two-phase outproj for the tail: kk=0 (hp0 ao, ready early) is
            # "armed" into aux psum during a diag-wait gap; kk=1 accumulates
            # + drains once hp1's transposed slice lands
            op_armed: dict = {}

            def op_arm(i, st, use_qk=False):
                mark(f"opa{i}.{st}")
                sloc = slice(128 * st, 128 * st + 128)
                if use_qk:
                    t = qkps.tile([128, 2, 512], F32, tag="qk", bufs=2, name="ps_oaq")
                    tiles = [t[:, 0, :], t[:, 1, :]]
                else:
                    tiles = [
                        auxps.tile([128, 512], F32, tag="aux", bufs=2, name="ps_oa")[:]
                        for _ in range(2)
                    ]
                for n2 in range(2):
                    nc.tensor.matmul(
                        tiles[n2],
                        ao_tiles[(i, 0)][:, sloc],
                        w_out_f[:, 512 * n2 : 512 * n2 + 512],
                        start=True,
                        stop=False,
                    )
                op_armed[(i, st)] = tiles

            def op_fin(i, st, split_dma=True, act_half=0):
                mark(f"opf{i}.{st}")
                ssl = slice(512 * i + 128 * st, 512 * i + 128 * st + 128)
                sloc = slice(128 * st, 128 * st + 128)
                tiles = op_armed.pop((i, st))
                ostage = ostpool.tile([128, 2, 512], BF16, tag="ost", name="ostage")
                for n2 in range(2):
                    ps_o = tiles[n2]
                    nc.tensor.matmul(
                        ps_o,
                        ao_tiles[(i, 1)][:, sloc],
                        w_out_f[:, D + 512 * n2 : D + 512 * n2 + 512],
                        start=False,
                        stop=True,
                    )
                    if n2 == act_half:
                        nc.scalar.copy(ostage[:, n2, :], ps_o)
                    else:
                        nc.vector.tensor_copy(ostage[:, n2, :], ps_o)
                    if split_dma:
                        (nc.scalar if n2 == 0 else nc.sync).dma_start(
                            outp[ssl, 512 * n2 : 512 * n2 + 512], ostage[:, n2, :]
                        )
                if not split_dma:
                    nc.sync.dma_start(
                        outp[ssl, :], ostage[:].rearrange("p a b -> p (a b)")
                    )

            # ---------------- attention ----------------
            ao_tiles: dict[tuple[int, int], object] = {}
            prefetched: dict = {}  # (i, hp) -> {j: phat}

            def qk_unit(i, hp, j):
                """qk matmul + exp (+mask) for unit (i, hp, j) -> phat.
                Diagonal blocks write their w = 512-128o valid columns
                LEFT-SHIFTED to offset 0 in ps_qk/phat (hardware-safe AP form
                for the exp; the causal strip is phat[:, :, 0:128])."""
                mark(f"qk{i}.{hp}.{j}")
                qt = qkT[2 * hp]
                kt = qkT[2 * hp + 1]
                dvr = causal and 4 * i <= j
                o = j - 4 * i if dvr else 0
                w = 512 - 128 * o
                ps_qk = qkps.tile([128, 2, 512], F32, tag="qk", bufs=2, name="ps_qk")
                for sl2 in range(2):
                    hb = 64 * sl2
                    nc.tensor.matmul(
                        ps_qk[:, sl2, 0:w],
                        kt[hb : hb + 64, 128 * j : 128 * j + 128],
                        qt[hb : hb + 64, 512 * i + 128 * o : 512 * i + 512],
                        start=True,
                        stop=True,
                    )
                phat = phatpool.tile([128, 2, 512], BF16, tag="phat", name="phat")
                nc.scalar.activation(
                    phat[:, :, 0:w], ps_qk[:, :, 0:w], AF.Exp,
                    scale=1.0 / math.sqrt(HD),
                )
                if dvr:
                    with tc.high_priority():
                        nc.vector.tensor_mul(
                            phat[:, :, 0:128], phat[:, :, 0:128], mask01_t[:]
                        )
                return phat

            def PFQ(i, hp, j):
                """Prefetch unit (i, hp, j)'s qk+exp from an earlier chunk's
                interleave -- pulls exp work into ACT-slack windows."""
                def f():
                    prefetched.setdefault((i, hp), {})[j] = qk_unit(i, hp, j)
                return f

            def make_attention(i, hp, interleave, tail=False):
                """Emit attention chunk i, head-pair hp. interleave = list of
                (slot, fn): fn fires after unit `slot` (-1 = before qk(0),
                -2 = right after qk(0) is issued).

                pv is FLIPPED: for each live query-slice o, psum group
                (sl2, o) = [128 queries, 65] accumulates over key blocks j;
                col 64 (ones in va) = softmax denominator per query. Group
                (sl2, o) stops at diagonal unit j = 4i+o (causal); right
                after, recip + per-partition normalize produce aoQ [q, f],
                and a DMA XBAR transpose writes ao[:, 128o:128o+128].

                STAGGER: group o's first pv matmul is deferred to unit o
                (catch-up burst of units 0..o there) so a new chunk's writes
                to psum region o never collide with the previous chunk's
                pending normalize reads of that region (pv_ps bufs=1).

                tail (last chunk, hp1): slices o>=2 transpose via PE (ident
                matmul into a spare qk-psum buffer + DVE copy) instead of the
                XBAR DMA, and normalize runs on DVE -- shorter tail chain."""
                nblk = 4 * i + 4 if causal else NT
                qt = qkT[2 * hp]
                kt = qkT[2 * hp + 1]
                # two pv tiles (o=0,1 / o=2,3) so a new chunk's early-region
                # writes only WAR against the previous chunk's early-region
                # reads (which finish long before its late ones)
                pvA = pvps.tile([128, 2, 2, HD + 1], F32, tag="pvA", bufs=1, name="ps_pvA")
                pvB = pvps.tile([128, 2, 2, HD + 1], F32, tag="pvB", bufs=1, name="ps_pvB")

                def pv_reg(sl2, o):
                    t = pvA if o < 2 else pvB
                    return t[:, sl2, o % 2, :]

                ao = aopool.tile([128, 512], BF16, tag="aot", name=f"ao_{i}_{hp}")
                ao_tiles[(i, hp)] = ao

                def reg_of(j):
                    dvr = causal and 4 * i <= j
                    o = j - 4 * i if dvr else 0
                    return dvr, o, slice(128 * o, 512)

                def diag_done(o):
                    """Group (*, o) just stopped: recip + normalize + XBAR
                    transpose into ao columns [128o, 128o+128)."""
                    mark(f"nz{i}.{hp}.{o}")
                    pe_tp = tail
                    pvt = pvA if o < 2 else pvB
                    rec128 = normpool.tile([128, 2, 1], F32, tag="rec128", bufs=4, name="rec128")
                    aoQ = aoqpool.tile([128, 2, HD], BF16, tag="aoq", name="aoQ")
                    with tc.high_priority():
                        nc.vector.reciprocal_approx_fast(
                            rec128[:], pvt[:, :, o % 2, HD : HD + 1]
                        )
                        # one DVE op for both heads: rec broadcast along hd
                        nc.vector.tensor_mul(
                            aoQ[:],
                            pvt[:, :, o % 2, 0:HD],
                            rec128[:].broadcast_to([128, 2, HD]),
                        )
                    if pe_tp:
                        psT = qkps.tile([128, 2, 1024], BF16, tag="qk", bufs=2, name="psT")
                        with tc.high_priority():
                            nc.tensor.transpose(
                                psT[:, 0, 0:128],
                                aoQ[:].rearrange("p a b -> p (a b)"),
                                ident_t[:],
                            )
                            if o >= 2:
                                # late tail: ACT is idle once the last exps
                                # drain; DVE is busy with ostage copies
                                nc.scalar.copy(
                                    ao[:, 128 * o : 128 * o + 128], psT[:, 0, 0:128]
                                )
                            else:
                                nc.vector.tensor_copy(
                                    ao[:, 128 * o : 128 * o + 128], psT[:, 0, 0:128]
                                )
                    else:
                        nc.sync.dma_start(
                            ao[:, 128 * o : 128 * o + 128],
                            aoQ[:].rearrange("p a b -> p (a b)"),
                            transpose=True,
                        )

                def fire(slot):
                    for sl, fn in interleave:
                        if sl == slot:
                            fn()

                def shift_of(jj):
                    return (jj - 4 * i) if (causal and jj >= 4 * i) else 0

                fire(-1)
                phats = dict(prefetched.pop((i, hp), {}))
                if 0 not in phats:
                    phats[0] = qk_unit(i, hp, 0)
                fire(-2)  # filler AFTER qk(0) is issued (exp chain started)
                stag = {o: [] for o in range(4)}
                # start=True zeroes the WHOLE 2KB psum bank (lazily, at each
                # byte's next matmul touch) -- so exactly ONE start per pv
                # bank per chunk; every other region gets its fresh start via
                # the bank's pending-zero on first touch, then accumulates.
                bank_started = {0: False, 1: False}
                for j in range(nblk):
                    if j + 1 < nblk and (j + 1) not in phats:
                        phats[j + 1] = qk_unit(i, hp, j + 1)
                    mark(f"pv{i}.{hp}.{j}")
                    dvr, o0, reg = reg_of(j)
                    lives = range(o0, 4) if causal else range(4)
                    for o in lives:
                        stag[o].append(j)
                        if j >= o:
                            for jj in stag[o]:
                                c0 = 128 * (o - shift_of(jj))
                                stop_j = (4 * i + o) if causal else (NT - 1)
                                for sl2 in range(2):
                                    bk = 0 if o < 2 else 1
                                    st = not bank_started[bk]
                                    bank_started[bk] = True
                                    nc.tensor.matmul(
                                        pv_reg(sl2, o),
                                        phats[jj][:, sl2, c0 : c0 + 128],
                                        va_t[:, jj, 2 * hp + sl2, :],
                                        start=st,
                                        stop=(jj == stop_j),
                                        skip_group_check=True,
                                    )
                            stag[o].clear()
                    if causal and dvr:
                        diag_done(o0)
                    fire(j)
                    if j >= 3:
                        for jj in [k for k in phats if k <= j]:
                            del phats[jj]
                if not causal:
                    for o in range(4):
                        diag_done(o)

            # ---------------- schedule ----------------
            if causal:
                # prelude: quarter-0 q-pair0 + k-pair0 projections + all v
                # (v first: w_v arrives before the trig tables)
                r0 = proj_group(0, 0)
                r1 = proj_group(0, 1)
                v_group(0, 0)
                proj_rot(0, 0, r0)
                v_group(0, 1)
                proj_rot(0, 1, r1)
                v_group(0, 2)
                v_group(0, 3)

                def G(q, mt, box, key):
                    def f():
                        box[key] = proj_group(q, mt)
                    return f

                def R(q, mt, box, key):
                    def f():
                        proj_rot(q, mt, box[key])
                    return f

                def V(q, st):
                    return lambda: v_group(q, st)

                def OP(i, st):
                    def f():
                        outproj_tile(i, st, [ao_tiles[(i, 0)], ao_tiles[(i, 1)]])
                    return f

                def XQ(q):
                    return lambda: issue_xq(q, nc.sync)

                bx: dict = {}
                items = {
                    (0, 0): [
                        (-2, G(0, 2, bx, "a")), (0, G(0, 3, bx, "b")),
                        (1, R(0, 2, bx, "a")), (2, R(0, 3, bx, "b")),
                    ],
                    (0, 1): [
                        (-2, XQ(2)),
                        (-2, G(1, 0, bx, "c")), (0, G(1, 1, bx, "d")),
                        (1, R(1, 0, bx, "c")), (2, V(1, 0)),
                        (3, R(1, 1, bx, "d")), (3, PFQ(1, 0, 0)),
                    ],
                    (1, 0): [
                        (-2, V(1, 1)),
                        (0, G(1, 2, bx, "e")), (1, V(1, 2)),
                        (2, R(1, 2, bx, "e")),
                        (4, G(1, 3, bx, "f")), (5, V(1, 3)),
                        (5, PFQ(1, 1, 0)),
                        (6, R(1, 3, bx, "f")), (7, PFQ(1, 1, 1)),
                    ],
                    (1, 1): [
                        (-2, XQ(3)),
                        (-2, G(2, 0, bx, "g")), (0, OP(0, 0)),
                        (2, R(2, 0, bx, "g")), (3, OP(0, 1)),
                        (4, G(2, 1, bx, "h")), (4, PFQ(2, 0, 0)),
                        (5, V(2, 0)), (5, PFQ(2, 0, 1)),
                        (6, R(2, 1, bx, "h")), (7, PFQ(2, 0, 2)),
                    ],
                    (2, 0): [
                        (-2, OP(0, 2)), (0, G(3, 0, bx, "m")),
                        (1, OP(0, 3)),
                        (2, G(2, 2, bx, "k")), (3, V(2, 1)),
                        (4, R(2, 2, bx, "k")), (5, R(3, 0, bx, "m")),
                        (6, G(2, 3, bx, "l")), (7, PFQ(2, 1, 0)),
                        (8, R(2, 3, bx, "l")), (8, V(2, 2)),
                        (9, PFQ(2, 1, 1)), (10, V(2, 3)),
                        (11, PFQ(2, 1, 2)),
                    ],
                    (2, 1): [
                        (-2, PFQ(3, 0, 0)), (0, PFQ(3, 0, 1)),
                        (2, PFQ(3, 0, 2)),
                        (3, G(3, 2, bx, "n")), (4, PFQ(3, 0, 3)),
                        (5, R(3, 2, bx, "n")), (6, PFQ(3, 0, 4)),
                        (7, V(3, 0)), (8, PFQ(3, 0, 5)), (9, V(3, 1)),
                        (10, PFQ(3, 0, 6)), (11, PFQ(3, 0, 7)),
                    ],
                    (3, 0): [
                        (-2, G(3, 1, bx, "o")), (0, R(3, 1, bx, "o")),
                        (1, OP(1, 0)), (2, V(3, 2)),
                        (4, OP(1, 1)), (6, V(3, 3)),
                        (8, OP(1, 2)), (10, OP(1, 3)),
                        (12, OP(2, 0)),
                    ],
                    (3, 1): [
                        (-2, OP(2, 2)), (2, G(3, 3, bx, "p")),
                        (4, OP(2, 3)), (6, R(3, 3, bx, "p")),
                        (8, OP(2, 1)),
                        (10, lambda: op_arm(3, 0)),
                        (12, lambda: op_fin(3, 0)),
                        (12, lambda: op_arm(3, 1)),
                        (14, lambda: op_fin(3, 1)),
                        (14, lambda: op_arm(3, 2)),
                        (15, lambda: op_fin(3, 2)),
                    ],
                }

                for i in range(NI):
                    make_attention(i, 0, items[(i, 0)])
                    make_attention(i, 1, items[(i, 1)], tail=(i == NI - 1))
                # tail: final outproj tile right after chunk 3 hp1's last slice
                outproj_tile(
                    3, 3, [ao_tiles[(3, 0)], ao_tiles[(3, 1)]], split_dma=True
                )
                if DEBUG_TAPS:
                    for mt in range(4):
                        nc.sync.dma_start(d_qkT[mt], qkT[mt][:])
                    for i in range(NI):
                        for hp in range(2):
                            nc.sync.dma_start(d_ao[i, hp], ao_tiles[(i, hp)][:])
            else:
                # non-causal: project everything first, then attention chunks
                # with outproj(i-1) interleaved.
                issue_xq(2, nc.sync)
                for q in range(NI):
                    if q == 2:
                        issue_xq(3, nc.sync)
                    rr = [proj_group(q, mt) for mt in range(4)]
                    for mt in range(4):
                        proj_rot(q, mt, rr[mt])
                    for st in range(4):
                        v_group(q, st)
                for i in range(NI):
                    its = []
                    if i > 0:
                        ao_pair = [ao_tiles[(i - 1, 0)], ao_tiles[(i - 1, 1)]]
                        for st in range(4):
                            its.append(
                                (2 * st, (lambda s, p: lambda: outproj_tile(i - 1, s, p))(st, ao_pair))
                            )
                    make_attention(i, 0, its)
                    make_attention(i, 1, [])
                ao_pair = [ao_tiles[(NI - 1, 0)], ao_tiles[(NI - 1, 1)]]
                for st in range(4):
                    outproj_tile(NI - 1, st, ao_pair)

    nc.compile()
    return nc


# --------------------------------------------------------------------------
# host-side: constants, sharding, assembly
# --------------------------------------------------------------------------

def _rope_tables():
    inv_freq = 1.0 / (ROPE_BASE ** (np.arange(0, HD, 2, dtype=np.float32) / HD))
    positions = np.arange(S, dtype=np.float32)
    freqs = np.outer(positions, inv_freq).astype(np.float32)     # [S, 32]
    emb = np.concatenate((freqs, freqs), axis=-1)                # [S, 64]
    cosT = np.cos(emb).T.astype(np.float32)                      # [64, S]
    sinT = np.sin(emb).T.astype(np.float32)
    cos2 = np.vstack([cosT, cosT]).astype(ml_dtypes.bfloat16)    # [128, S]
    sin2 = np.vstack([sinT, sinT]).astype(ml_dtypes.bfloat16)
    return cos2, sin2


def _rot_lhsT():
    # rotate_half (interleaved): rot[2i] = -x[2i+1], rot[2i+1] = x[2i]
    P = np.zeros((128, 128), np.float32)
    for base in (0, 64):
        for i2 in range(HD // 2):
            P[base + 2 * i2, base + 2 * i2 + 1] = -1.0
            P[base + 2 * i2 + 1, base + 2 * i2] = 1.0
    return np.ascontiguousarray(P.T).astype(ml_dtypes.bfloat16)


_CACHE: dict = {}


def _get_nc(causal: bool):
    key = ("nc", causal)
    if key not in _CACHE:
        _CACHE[key] = build_nc(causal)
    return _CACHE[key]


def _classify_mask(mask: np.ndarray) -> str:
    m = np.asarray(mask).reshape(S, S)
    if np.array_equal(m != 0, np.tril(np.ones((S, S), bool))):
        return "causal"
    if np.all(m != 0):
        return "full"
    return "other"


def make_in_maps(x, w_qkv, w_out):
    """Build the 8 per-core input dicts."""
    cos2, sin2 = _rope_tables()
    rotP = _rot_lhsT()
    m01 = (np.arange(128)[:, None] <= np.arange(128)[None, :]).astype(np.float32)
    mask01 = np.ascontiguousarray(
        np.stack([m01, m01], axis=1).reshape(128, 256)
    ).astype(ml_dtypes.bfloat16)

    w3 = np.asarray(w_qkv).reshape(D, 3, H, HD)   # [D, {q,k,v}, H, hd]
    wo = np.asarray(w_out)                        # [D, D]; rows indexed [h, hd]
    xT = [
        np.ascontiguousarray(np.asarray(x)[b].T).astype(ml_dtypes.bfloat16)
        for b in range(B)
    ]  # [D, S]

    in_maps = []
    for c in range(8):
        b, hg = divmod(c, 4)
        hs = [4 * hg + i for i in range(HPC)]
        # w_qk cols: [q_h0, q_h1, k_h0, k_h1, q_h2, q_h3, k_h2, k_h3]
        wqk_cols = []
        for pair in range(2):
            for t in range(2):  # 0 = q, 1 = k
                for hh in (hs[2 * pair], hs[2 * pair + 1]):
                    wqk_cols.append(w3[:, t, hh, :])
        w_qk_c = np.ascontiguousarray(np.concatenate(wqk_cols, axis=1)).astype(
            ml_dtypes.bfloat16
        )  # [D, 512]
        w_v_c = np.ascontiguousarray(
            np.concatenate([w3[:, 2, hh, :] for hh in hs], axis=1)
        ).astype(ml_dtypes.bfloat16)  # [D, 256]
        w_out_c = np.ascontiguousarray(
            np.concatenate([wo[HD * hh : HD * hh + HD, :] for hh in hs], axis=0)
        ).astype(ml_dtypes.bfloat16)  # [256, D]
        in_maps.append(
            {
                "xT": xT[b],
                "w_qk": w_qk_c,
                "w_v": w_v_c,
                "w_out": w_out_c,
                "cos2": cos2,
                "sin2": sin2,
                "rotP": rotP,
                "ident": np.eye(128, dtype=np.float32).astype(ml_dtypes.bfloat16),
                "mask01": mask01,
            }
        )
    return in_maps


def _reference_numpy(x, mask, w_qkv, w_out):
    """Exact fallback for non-causal, non-full masks (slow, host-side)."""
    x = np.asarray(x, np.float32)
    qkv = (x @ w_qkv).reshape(B, S, 3, H, HD)
    qkv = np.transpose(qkv, (2, 0, 3, 1, 4))
    q, k, v = qkv[0], qkv[1], qkv[2]
    inv_freq = 1.0 / (ROPE_BASE ** (np.arange(0, HD, 2, dtype=np.float32) / HD))
    freqs = np.outer(np.arange(S, dtype=np.float32), inv_freq)
    emb = np.concatenate((freqs, freqs), axis=-1)
    cos = np.cos(emb)[None, None]
    sin = np.sin(emb)[None, None]

    def rot(t):
        t1 = t[..., ::2]
        t2 = t[..., 1::2]
        return np.stack((-t2, t1), axis=-1).reshape(t.shape)

    q = q * cos + rot(q) * sin
    k = k * cos + rot(k) * sin
    attn = np.einsum("bhsd,bhtd->bhst", q, k) / math.sqrt(HD)
    m = np.asarray(mask).reshape(1, 1, S, S)
    attn = np.where(m == 0, -np.inf, attn)
    attn = attn - attn.max(-1, keepdims=True)
    np.exp(attn, out=attn)
    attn /= attn.sum(-1, keepdims=True)
    out = np.einsum("bhst,bhtd->bhsd", attn, v)
    out = np.transpose(out, (0, 2, 1, 3)).reshape(B, S, D)
    return (out @ w_out).astype(np.float32)


class Runner:
    """Cached jitted SPMD runner (mirrors bass2jax.run_bass_via_pjrt)."""

    def __init__(self, nc, n_cores: int = 8):
        import jax
        import concourse.mybir as _mybir
        from concourse import bass2jax
        from jax.experimental.shard_map import shard_map
        from jax.sharding import Mesh, PartitionSpec

        bass2jax.install_neuronx_cc_hook()
        self.jax = jax
        self.n_cores = n_cores
        self._nc = nc
        in_names, out_names, out_avals, zero_outs = [], [], [], []
        for alloc in nc.m.functions[0].allocations:
            if not isinstance(alloc, _mybir.MemoryLocationSet):
                continue
            name = alloc.memorylocations[0].name
            if alloc.kind == "ExternalInput":
                in_names.append(name)
            elif alloc.kind == "ExternalOutput":
                out_names.append(name)
                shape = tuple(alloc.tensor_shape)
                dtype = _mybir.dt.np(alloc.dtype)
                out_avals.append(jax.core.ShapedArray(shape, dtype))
                zero_outs.append(np.zeros(shape, dtype))
        self.in_names = list(in_names)
        self.out_names = out_names
        self.out_avals = out_avals
        self.zero_outs = zero_outs
        all_names = in_names + out_names

        def _body(*args):
            outs = bass2jax._bass_exec_p.bind(
                *args,
                out_avals=tuple(out_avals),
                in_names=tuple(all_names),
                out_names=tuple(out_names),
                lowering_input_output_aliases=(),
                sim_require_finite=True,
                sim_require_nnan=True,
                nc=nc,
            )
            return tuple(outs)

        devices = jax.devices()[:n_cores]
        self.mesh = Mesh(np.asarray(devices), ("core",))
        n_args = len(all_names)
        self.sharded = jax.jit(
            shard_map(
                _body,
                mesh=self.mesh,
                in_specs=(PartitionSpec("core"),) * n_args,
                out_specs=(PartitionSpec("core"),) * len(out_names),
                check_rep=False,
            )
        )

    def concat_inputs(self, in_maps):
        cols = []
        for name in self.in_names:
            if name == "partition_id":
                cols.append(
                    np.arange(self.n_cores, dtype=np.uint32).reshape(self.n_cores, 1)
                )
            else:
                cols.append(
                    np.concatenate([np.asarray(m[name]) for m in in_maps], axis=0)
                )
        return cols

    def device_put(self, concat_in):
        from jax.sharding import NamedSharding, PartitionSpec

        sh = NamedSharding(self.mesh, PartitionSpec("core"))
        args = concat_in + [
            np.zeros((self.n_cores * z.shape[0], *z.shape[1:]), z.dtype)
            for z in self.zero_outs
        ]
        return [self.jax.device_put(a, sh) for a in args]

    def run_dev(self, dev_args):
        return self.sharded(*dev_args)

    def make_bench(self, n_reps: int):
        import jax
        from concourse import bass2jax
        from jax.experimental.shard_map import shard_map
        from jax.sharding import Mesh, PartitionSpec

        nc = self._nc
        out_avals = self.out_avals
        all_names = self.in_names + self.out_names
        out_names = self.out_names

        def _body(*args):
            outs = None
            for _ in range(n_reps):
                outs = bass2jax._bass_exec_p.bind(
                    *args,
                    out_avals=tuple(out_avals),
                    in_names=tuple(all_names),
                    out_names=tuple(out_names),
                    lowering_input_output_aliases=(),
                    sim_require_finite=True,
                    sim_require_nnan=True,
                    nc=nc,
                )
            return tuple(outs)

        n_args = len(all_names)
        return jax.jit(
            shard_map(
                _body,
                mesh=self.mesh,
                in_specs=(PartitionSpec("core"),) * n_args,
                out_specs=(PartitionSpec("core"),) * len(out_names),
                check_rep=False,
            )
        )

    def run(self, in_maps):
        dev_args = self.device_put(self.concat_inputs(in_maps))
        out_arrs = self.sharded(*dev_args)
        outs = []
        for c in range(self.n_cores):
            outs.append(
                {
                    name: np.asarray(out_arrs[i]).reshape(
                        self.n_cores, *self.out_avals[i].shape
                    )[c]
                    for i, name in enumerate(self.out_names)
                }
            )
        return outs


def _get_runner(causal: bool) -> Runner:
    key = ("runner", causal)
    if key not in _CACHE:
        _CACHE[key] = Runner(_get_nc(causal))
    return _CACHE[key]


def run_spmd(in_maps, causal: bool = True, **kw):
    nc = _get_nc(causal)
    return run_bass_kernel_spmd(nc, in_maps, core_ids=list(range(8)), **kw)


def kernel(x, mask, w_qkv, w_out):
    kind = _classify_mask(mask)
    if kind != "causal":
        # exact host fallback for non-causal masks (the v4 schedule is
        # specialized for the causal diagonal structure)
        return _reference_numpy(x, mask, w_qkv, w_out)
    in_maps = make_in_maps(x, w_qkv, w_out)
    res = run_spmd(in_maps, causal=True)
    out = np.zeros((B, S, D), np.float32)
    for c in range(8):
        out[c // 4] += np.asarray(res.results[c]["outp"]).astype(np.float32)
    return out


if __name__ == "__main__":
    rng = np.random.default_rng(0)
    x = rng.standard_normal((B, S, D)).astype(np.float32)
    mask = np.tril(np.ones((S, S), np.int32)).reshape(1, 1, S, S)
    w_qkv = (rng.standard_normal((D, 3 * D)) * 0.02).astype(np.float32)
    w_out = (rng.standard_normal((D, D)) * 0.02).astype(np.float32)
    got = kernel(x, mask, w_qkv, w_out)
    print("kernel ran, out shape", got.shape)


# revision 104
# speedup vs baseline: 1.0016x; 1.0016x over previous
"""Multi-head attention (RoPE, causal) Trainium2 Bass kernel, v4.

Problem: nn_MultiHeadAttention_62431644615193
  x: [2, 2048, 1024] f32, mask: causal tril, w_qkv: [1024, 3072], w_out: [1024, 1024]

Sharding: 8 cores = batch(2) x head-groups(4 heads each). Each core emits a
bf16 partial [2048, 1024] (its heads through w_out rows); host sums 4
partials per batch in f32.

v4 design (157.9us -> 126.7us cost-model):
  - pv matmuls FLIPPED: out[128 queries, hd+1] per (query-slice o, head)
    accumulation region (lhsT = phat 128x128 slice, rhs = va[keys, 65]).
    M=65 -> 128 saves ~14us PE; the softmax denominator (ones column of
    va) lands at psum col 64 = per-partition, so normalization is one
    per-partition DVE mul (rec broadcast along hd). Kills v2's gpsimd
    partition broadcasts, den copies, and un-staging.
  - PSUM START IS BANK-GRANULAR: start=True marks the whole 2KB bank
    pending-zero (lazily applied at each byte's next matmul touch), so
    with 4 accumulation regions packed per pv bank exactly ONE matmul per
    bank per chunk carries start=True; every other region gets its fresh
    start via the bank's pending-zero at first touch, then accumulates.
  - STAGGER: region o's first pv matmul is deferred to unit o (catch-up
    burst) so a new chunk's writes never collide with the previous
    chunk's pending normalize reads (pvA: o=0,1 / pvB: o=2,3, bufs=1).
  - ao [q, f] -> [f, q] via DMA XBAR transpose (14ns/16x128 tile, off all
    compute engines); last chunk hp1 uses PE ident-transposes instead
    (shorter tail chain). outproj consumes ao tiles unchanged.
  - GPSIMD cannot access PSUM: all psum drains are DVE/ACT. ACT takes
    low-bass_priority copies (exp always wins); DVE reads that release
    aux psum (qk_raw, rotsin, va) are high-priority.
  - Cross-chunk qk+exp PREFETCH (PFQ) pulls exp work into ACT-slack
    windows of earlier chunks (chunk 3's first 5 units run inside 2,1).
  - Filler schedule: outproj(i) deferred 1-2 chunks (aopool bufs=8);
    quarter-3 k-proj/v-proj deferred into chunk 3; outproj tail split-DMA
    on alternating SP/ACT queues.
  - PSUM: qk 2x[128,2,512] (4 banks) + pvA/pvB [128,2,2,65] (2) + aux
    2x[128,512] (2) = 8 banks exactly.
"""

import math
import os

import numpy as np
import ml_dtypes

DEBUG_TAPS = bool(os.environ.get("KERNEL_DEBUG_TAPS"))

import concourse.bass as bass
import concourse.tile as tile
from concourse import bacc
import concourse.mybir as mybir
from concourse.bass_utils import run_bass_kernel_spmd

B, S, D = 2, 2048, 1024
H = 16
HD = D // H          # 64
HPC = H // 4         # 4 heads per core
ROPE_BASE = 10000.0

F32 = mybir.dt.float32
F32R = mybir.dt.float32r
BF16 = mybir.dt.bfloat16
AF = mybir.ActivationFunctionType

NT = S // 128        # 16 t-blocks
NI = 4               # 512-wide s-chunks

SECTIONS: list = []  # (start_instruction_id, label) in emit order


def build_nc(causal: bool = True):
    nc = bacc.Bacc("TRN2", target_bir_lowering=False, debug=False, num_devices=8)
    SECTIONS.clear()

    def mark(label):
        SECTIONS.append((nc.next_id(), label))

    xT = nc.dram_tensor("xT", [D, S], BF16, kind="ExternalInput")
    w_qk = nc.dram_tensor("w_qk", [D, 8 * HD], BF16, kind="ExternalInput")
    w_v = nc.dram_tensor("w_v", [D, 4 * HD], BF16, kind="ExternalInput")
    w_out = nc.dram_tensor("w_out", [4 * HD, D], BF16, kind="ExternalInput")
    cos2 = nc.dram_tensor("cos2", [128, S], BF16, kind="ExternalInput")
    sin2 = nc.dram_tensor("sin2", [128, S], BF16, kind="ExternalInput")
    rotP = nc.dram_tensor("rotP", [128, 128], BF16, kind="ExternalInput")
    ident = nc.dram_tensor("ident", [128, 128], BF16, kind="ExternalInput")
    mask01 = nc.dram_tensor("mask01", [128, 2 * 128], BF16, kind="ExternalInput")
    outp = nc.dram_tensor("outp", [S, D], BF16, kind="ExternalOutput")
    if DEBUG_TAPS:
        d_qkT = nc.dram_tensor("d_qkT", [4, 128, S], BF16, kind="ExternalOutput")
        d_ao = nc.dram_tensor("d_ao", [NI, 2, 128, 512], BF16, kind="ExternalOutput")

    with tile.TileContext(nc) as tc:
        with (
            tc.tile_pool(name="const", bufs=1) as cpool,
            tc.tile_pool(name="qkT", bufs=1) as qkTpool,
            tc.tile_pool(name="va", bufs=1) as vapool,
            tc.tile_pool(name="xq", bufs=2) as xqpool,
            tc.tile_pool(name="qkraw", bufs=2) as qkrawpool,
            tc.tile_pool(name="ropescratch", bufs=2) as rspool,
            tc.tile_pool(name="phat", bufs=15) as phatpool,
            tc.tile_pool(name="norm", bufs=4) as normpool,
            tc.tile_pool(name="aoq", bufs=10) as aoqpool,
            tc.tile_pool(name="attn_out", bufs=8) as aopool,
            tc.tile_pool(name="outstage", bufs=4) as ostpool,
            tc.tile_pool(name="psqk", bufs=1, space="PSUM") as qkps,
            tc.tile_pool(name="pspv", bufs=1, space="PSUM") as pvps,
            tc.tile_pool(name="psaux", bufs=1, space="PSUM") as auxps,
        ):
            # ---------------- constants (consolidated tiles) ----------------
            w_qk_t = cpool.tile([128, 8, 8 * HD], BF16, name="wqkt", tag="wqkt")
            w_v_t = cpool.tile([128, 8, 4 * HD], BF16, name="wvt", tag="wvt")
            w_out_t = cpool.tile([128, 2, D], BF16, name="woutt", tag="woutt")
            rotP_t = cpool.tile([128, 128], BF16)
            ident_t = cpool.tile([128, 128], BF16, name="ident_t", tag="ident_t")
            cos_t = cpool.tile([128, S], BF16, name="cos_t", tag="cos_t")
            sin_t = cpool.tile([128, S], BF16, name="sin_t", tag="sin_t")
            mask01_t = cpool.tile([128, 2, 128], BF16)

            # --- startup DMA plan: batched transfers (per-DMA overhead is
            # ~0.9us), consumption-ordered, alternating SP/ACT queues so the
            # serialized transfer pipe matches consumption order.
            def dd_slab(dram, a, b, cols=None):
                """dram rows [128a, 128b) as [128, b-a, cols] slab."""
                sl = dram[128 * a : 128 * b, :] if cols is None else dram[128 * a : 128 * b, cols]
                return sl.rearrange("(dd p) s -> p dd s", p=128)

            xq_tiles: dict[int, object] = {}

            def xq_first(q):
                xq = xqpool.tile([128, 8, 512], BF16, tag="xq", name=f"xq{q}")
                xq_tiles[q] = xq
                return xq

            def issue_xq(q, eng, split=(4,)):
                xq = xq_first(q)
                s_sl = slice(512 * q, 512 * q + 512)
                lo = 0
                for n in (*split, 8):
                    if n > lo:
                        eng.dma_start(xq[:, lo:n, :], dd_slab(xT, lo, n, s_sl))
                    lo = n

            va_t = vapool.tile([128, NT, 4, HD + 1], BF16)
            nc.gpsimd.memset(va_t[:, :, :, HD : HD + 1], 1.0)
            # prelude needs only w_qk cols 0:256 (q-pair0 + k-pair0); the
            # second half (cols 256:512, for pg(0,2)/pg(0,3) in chunk 0)
            # follows after the prelude-critical transfers
            cA = slice(0, 256)
            nc.sync.dma_start(w_qk_t[:, 0:2, cA], dd_slab(w_qk, 0, 2, cA))
            nc.scalar.dma_start(
                xq_first(0)[:, 0:2, :], dd_slab(xT, 0, 2, slice(0, 512))
            )
            nc.sync.dma_start(w_qk_t[:, 2:4, cA], dd_slab(w_qk, 2, 4, cA))
            nc.scalar.dma_start(
                xq_tiles[0][:, 2:4, :], dd_slab(xT, 2, 4, slice(0, 512))
            )
            nc.sync.dma_start(w_qk_t[:, 4:8, cA], dd_slab(w_qk, 4, 8, cA))
            nc.scalar.dma_start(
                xq_tiles[0][:, 4:8, :], dd_slab(xT, 4, 8, slice(0, 512))
            )
            nc.scalar.dma_start(rotP_t[:], rotP[:])
            nc.sync.dma_start(w_v_t[:, 0:4, :], dd_slab(w_v, 0, 4))
            nc.scalar.dma_start(cos_t[:, 0:512], cos2[:, 0:512])
            nc.sync.dma_start(w_v_t[:, 4:8, :], dd_slab(w_v, 4, 8))
            nc.scalar.dma_start(sin_t[:, 0:512], sin2[:, 0:512])
            cB = slice(256, 512)
            nc.sync.dma_start(w_qk_t[:, 0:8, cB], dd_slab(w_qk, 0, 8, cB))
            nc.sync.dma_start(
                mask01_t[:], mask01[:].rearrange("p (b s) -> p b s", b=2)
            )
            issue_xq(1, nc.sync)
            nc.scalar.dma_start(cos_t[:, 512:1024], cos2[:, 512:1024])
            nc.scalar.dma_start(sin_t[:, 512:1024], sin2[:, 512:1024])
            nc.sync.dma_start(cos_t[:, 1024:2048], cos2[:, 1024:2048])
            nc.sync.dma_start(sin_t[:, 1024:2048], sin2[:, 1024:2048])
            nc.sync.dma_start(
                w_out_t[:], w_out[:].rearrange("(kk p) s -> p kk s", p=128)
            )
            nc.scalar.dma_start(ident_t[:], ident[:])

            w_out_f = w_out_t[:].rearrange("p a b -> p (a b)")
            # rotated qT/kT (bf16): [q_h0;q_h1], [k_h0;k_h1], [q_h2;q_h3], [k_h2;k_h3]
            qkT = [qkTpool.tile([128, S], BF16, name=f"qkT{i}", tag=f"qkT{i}") for i in range(4)]

            # low-priority marker: the tile scheduler's heap picks lowest
            # bass_priority among READY instructions, so a large value makes
            # an op fill engine-idle gaps instead of delaying critical work
            _low_ctr = [1 << 20]

            def low_pri(bi):
                _low_ctr[0] += 1
                bi.ins.bass_priority = _low_ctr[0]
                return bi

            # ---------------- projection pieces ----------------
            def proj_group(q, mt):
                """qk-projection matmul group; returns qk_raw SBUF copy."""
                mark(f"pg{q}.{mt}")
                xq = xq_tiles[q]
                ps = auxps.tile([128, 512], F32, tag="aux", bufs=2, name="ps_g")
                for dd in range(8):
                    nc.tensor.matmul(
                        ps[:],
                        w_qk_t[:, dd, 128 * mt : 128 * mt + 128],
                        xq[:, dd, :],
                        start=(dd == 0),
                        stop=(dd == 7),
                    )
                qk_raw = qkrawpool.tile([128, 512], BF16, tag="qkraw", name="qk_raw")
                # PSUM reads are DVE/ACT-only (GPSIMD cannot access PSUM).
                # Quarter 1 runs while ACT has exp slack -> ACT low-pri;
                # quarter 0 (prelude; ACT SEQ is busy with DMA configs) and
                # later quarters go on DVE.
                if q == 1:
                    low_pri(nc.scalar.copy(qk_raw[:], ps[:]))
                else:
                    # high-pri: this read releases the aux psum buffer
                    with tc.high_priority():
                        nc.vector.tensor_copy(qk_raw[:], ps[:])
                return qk_raw

            def proj_rot(q, mt, qk_raw):
                """RoPE combine -> qkT[mt][:, quarter q] (bf16). rotate_half
                via a PE permutation matmul."""
                mark(f"rot{q}.{mt}")
                s_sl = slice(512 * q, 512 * q + 512)
                psr = auxps.tile([128, 512], F32, tag="aux", bufs=2, name="psr")
                nc.tensor.matmul(psr[:], rotP_t[:], qk_raw[:], start=True, stop=True)
                rotsin = rspool.tile([128, 512], BF16, tag="rs", name="rotsin")
                # high-pri: this read releases the aux psum buffer
                with tc.high_priority():
                    nc.vector.tensor_mul(rotsin[:], psr[:], sin_t[:, s_sl])
                qkcos = rspool.tile([128, 512], BF16, tag="qkcos", name="qkcos")
                nc.vector.tensor_mul(qkcos[:], qk_raw[:], cos_t[:, s_sl])
                nc.vector.tensor_add(qkT[mt][:, s_sl], qkcos[:], rotsin[:])

            def v_group(q, st):
                """v-projection for t-block j = 4q+st -> va (bf16)."""
                mark(f"vg{q}.{st}")
                j = 4 * q + st
                xq = xq_tiles[q]
                psv = auxps.tile([128, 4 * HD], F32, tag="aux", bufs=2, name="psv")
                for dd in range(8):
                    nc.tensor.matmul(
                        psv[:],
                        xq[:, dd, 128 * st : 128 * st + 128],
                        w_v_t[:, dd, :],
                        start=(dd == 0),
                        stop=(dd == 7),
                    )
                # high-pri: this read releases the aux psum buffer
                with tc.high_priority():
                    nc.vector.tensor_copy(
                        va_t[:, j, :, 0:HD], psv[:].rearrange("p (h c) -> p h c", h=4)
                    )

            # ---------------- outproj ----------------
            def outproj_tile(i, st, ao_pair, split_dma=False, use_qk_psum=False):
                mark(f"op{i}.{st}")
                ssl = slice(512 * i + 128 * st, 512 * i + 128 * st + 128)
                sloc = slice(128 * st, 128 * st + 128)
                ostage = ostpool.tile([128, 2, 512], BF16, tag="ost", name="ostage")
                # tail: the qk psum bufs are dead after the last unit; borrow
                # one per op tile so the final ops skip aux-psum recycling
                ps_pair = (
                    qkps.tile([128, 2, 512], F32, tag="qk", bufs=2, name="ps_ot")
                    if use_qk_psum else None
                )
                for n2 in range(2):
                    ps_o = (
                        ps_pair[:, n2, :] if ps_pair is not None
                        else auxps.tile([128, 512], F32, tag="aux", bufs=2, name="ps_o")[:]
                    )
                    for kk in range(2):
                        nc.tensor.matmul(
                            ps_o[:],
                            ao_pair[kk][:, sloc],
                            w_out_f[:, D * kk + 512 * n2 : D * kk + 512 * n2 + 512],
                            start=(kk == 0),
                            stop=(kk == 1),
                        )
                    # PSUM drains: ACT (low-pri, exp-slack windows) only for
                    # chunk-0 tiles and the tail (ACT idle there); chunk 1-3
                    # tiles fire inside ACT-saturated chunk 3 -> DVE
                    if n2 == 0 and (i == 0 or i == 3 or split_dma):
                        c = nc.scalar.copy(ostage[:, 0, :], ps_o[:])
                        if i == 0 and not split_dma:
                            low_pri(c)
                    else:
                        nc.vector.tensor_copy(ostage[:, n2, :], ps_o[:])
                    if split_dma:
                        # tail: per-half DMA on alternating SEQ queues so the
                        # configs overlap and the first transfer overlaps the
                        # second half's matmuls + copy
                        (nc.scalar if n2 == 0 else nc.sync).dma_start(
                            outp[ssl, 512 * n2 : 512 * n2 + 512], ostage[:, n2, :]
                        )
                if not split_dma:
                    nc.sync.dma_start(
                        outp[ssl, :], ostage[:].rearrange("p a b -> p (a b)")
                    )

            # two-phase outproj for the tail: kk=0 (hp0 ao, ready early) is
            # "armed" into aux psum during a diag-wait gap; kk=1 accumulates
            # + drains once hp1's transposed slice lands
            op_armed: dict = {}

            def op_arm(i, st, use_qk=False):
                mark(f"opa{i}.{st}")
                sloc = slice(128 * st, 128 * st + 128)
                if use_qk:
                    t = qkps.tile([128, 2, 512], F32, tag="qk", bufs=2, name="ps_oaq")
                    tiles = [t[:, 0, :], t[:, 1, :]]
                else:
                    tiles = [
                        auxps.tile([128, 512], F32, tag="aux", bufs=2, name="ps_oa")[:]
                        for _ in range(2)
                    ]
                for n2 in range(2):
                    nc.tensor.matmul(
                        tiles[n2],
                        ao_tiles[(i, 0)][:, sloc],
                        w_out_f[:, 512 * n2 : 512 * n2 + 512],
                        start=True,
                        stop=False,
                    )
                op_armed[(i, st)] = tiles

            def op_fin(i, st, split_dma=True, act_half=0):
                mark(f"opf{i}.{st}")
                ssl = slice(512 * i + 128 * st, 512 * i + 128 * st + 128)
                sloc = slice(128 * st, 128 * st + 128)
                tiles = op_armed.pop((i, st))
                ostage = ostpool.tile([128, 2, 512], BF16, tag="ost", name="ostage")
                for n2 in range(2):
                    ps_o = tiles[n2]
                    nc.tensor.matmul(
                        ps_o,
                        ao_tiles[(i, 1)][:, sloc],
                        w_out_f[:, D + 512 * n2 : D + 512 * n2 + 512],
                        start=False,
                        stop=True,
                    )
                    if n2 == act_half:
                        nc.scalar.copy(ostage[:, n2, :], ps_o)
                    else:
                        nc.vector.tensor_copy(ostage[:, n2, :], ps_o)
                    if split_dma:
                        (nc.scalar if n2 == 0 else nc.sync).dma_start(
                            outp[ssl, 512 * n2 : 512 * n2 + 512], ostage[:, n2, :]
                        )
                if not split_dma:
                    nc.sync.dma_start(
                        outp[ssl, :], ostage[:].rearrange("p a b -> p (a b)")
                    )

            # ---------------- attention ----------------
            ao_tiles: dict[tuple[int, int], object] = {}
            prefetched: dict = {}  # (i, hp) -> {j: phat}

            def qk_unit(i, hp, j):
                """qk matmul + exp (+mask) for unit (i, hp, j) -> phat.
                Diagonal blocks write their w = 512-128o valid columns
                LEFT-SHIFTED to offset 0 in ps_qk/phat (hardware-safe AP form
                for the exp; the causal strip is phat[:, :, 0:128])."""
                mark(f"qk{i}.{hp}.{j}")
                qt = qkT[2 * hp]
                kt = qkT[2 * hp + 1]
                dvr = causal and 4 * i <= j
                o = j - 4 * i if dvr else 0
                w = 512 - 128 * o
                ps_qk = qkps.tile([128, 2, 512], F32, tag="qk", bufs=2, name="ps_qk")
                for sl2 in range(2):
                    hb = 64 * sl2
                    nc.tensor.matmul(
                        ps_qk[:, sl2, 0:w],
                        kt[hb : hb + 64, 128 * j : 128 * j + 128],
                        qt[hb : hb + 64, 512 * i + 128 * o : 512 * i + 512],
                        start=True,
                        stop=True,
                    )
                phat = phatpool.tile([128, 2, 512], BF16, tag="phat", name="phat")
                nc.scalar.activation(
                    phat[:, :, 0:w], ps_qk[:, :, 0:w], AF.Exp,
                    scale=1.0 / math.sqrt(HD),
                )
                if dvr:
                    with tc.high_priority():
                        nc.vector.tensor_mul(
                            phat[:, :, 0:128], phat[:, :, 0:128], mask01_t[:]
                        )
                return phat

            def PFQ(i, hp, j):
                """Prefetch unit (i, hp, j)'s qk+exp from an earlier chunk's
                interleave -- pulls exp work into ACT-slack windows."""
                def f():
                    prefetched.setdefault((i, hp), {})[j] = qk_unit(i, hp, j)
                return f

            def make_attention(i, hp, interleave, tail=False):
                """Emit attention chunk i, head-pair hp. interleave = list of
                (slot, fn): fn fires after unit `slot` (-1 = before qk(0),
                -2 = right after qk(0) is issued).

                pv is FLIPPED: for each live query-slice o, psum group
                (sl2, o) = [128 queries, 65] accumulates over key blocks j;
                col 64 (ones in va) = softmax denominator per query. Group
                (sl2, o) stops at diagonal unit j = 4i+o (causal); right
                after, recip + per-partition normalize produce aoQ [q, f],
                and a DMA XBAR transpose writes ao[:, 128o:128o+128].

                STAGGER: group o's first pv matmul is deferred to unit o
                (catch-up burst of units 0..o there) so a new chunk's writes
                to psum region o never collide with the previous chunk's
                pending normalize reads of that region (pv_ps bufs=1).

                tail (last chunk, hp1): slices o>=2 transpose via PE (ident
                matmul into a spare qk-psum buffer + DVE copy) instead of the
                XBAR DMA, and normalize runs on DVE -- shorter tail chain."""
                nblk = 4 * i + 4 if causal else NT
                qt = qkT[2 * hp]
                kt = qkT[2 * hp + 1]
                # two pv tiles (o=0,1 / o=2,3) so a new chunk's early-region
                # writes only WAR against the previous chunk's early-region
                # reads (which finish long before its late ones)
                pvA = pvps.tile([128, 2, 2, HD + 1], F32, tag="pvA", bufs=1, name="ps_pvA")
                pvB = pvps.tile([128, 2, 2, HD + 1], F32, tag="pvB", bufs=1, name="ps_pvB")

                def pv_reg(sl2, o):
                    t = pvA if o < 2 else pvB
                    return t[:, sl2, o % 2, :]

                ao = aopool.tile([128, 512], BF16, tag="aot", name=f"ao_{i}_{hp}")
                ao_tiles[(i, hp)] = ao

                def reg_of(j):
                    dvr = causal and 4 * i <= j
                    o = j - 4 * i if dvr else 0
                    return dvr, o, slice(128 * o, 512)

                def diag_done(o):
                    """Group (*, o) just stopped: recip + normalize + XBAR
                    transpose into ao columns [128o, 128o+128)."""
                    mark(f"nz{i}.{hp}.{o}")
                    pe_tp = tail
                    pvt = pvA if o < 2 else pvB
                    rec128 = normpool.tile([128, 2, 1], F32, tag="rec128", bufs=4, name="rec128")
                    aoQ = aoqpool.tile([128, 2, HD], BF16, tag="aoq", name="aoQ")
                    with tc.high_priority():
                        nc.vector.reciprocal_approx_fast(
                            rec128[:], pvt[:, :, o % 2, HD : HD + 1]
                        )
                        # one DVE op for both heads: rec broadcast along hd
                        nc.vector.tensor_mul(
                            aoQ[:],
                            pvt[:, :, o % 2, 0:HD],
                            rec128[:].broadcast_to([128, 2, HD]),
                        )
                    if pe_tp:
                        psT = qkps.tile([128, 2, 1024], BF16, tag="qk", bufs=2, name="psT")
                        with tc.high_priority():
                            nc.tensor.transpose(
                                psT[:, 0, 0:128],
                                aoQ[:].rearrange("p a b -> p (a b)"),
                                ident_t[:],
                            )
                            if o >= 2:
                                # late tail: ACT is idle once the last exps
                                # drain; DVE is busy with ostage copies
                                nc.scalar.copy(
                                    ao[:, 128 * o : 128 * o + 128], psT[:, 0, 0:128]
                                )
                            else:
                                nc.vector.tensor_copy(
                                    ao[:, 128 * o : 128 * o + 128], psT[:, 0, 0:128]
                                )
                    else:
                        nc.sync.dma_start(
                            ao[:, 128 * o : 128 * o + 128],
                            aoQ[:].rearrange("p a b -> p (a b)"),
                            transpose=True,
                        )

                def fire(slot):
                    for sl, fn in interleave:
                        if sl == slot:
                            fn()

                def shift_of(jj):
                    return (jj - 4 * i) if (causal and jj >= 4 * i) else 0

                fire(-1)
                phats = dict(prefetched.pop((i, hp), {}))
                if 0 not in phats:
                    phats[0] = qk_unit(i, hp, 0)
                fire(-2)  # filler AFTER qk(0) is issued (exp chain started)
                stag = {o: [] for o in range(4)}
                # start=True zeroes the WHOLE 2KB psum bank (lazily, at each
                # byte's next matmul touch) -- so exactly ONE start per pv
                # bank per chunk; every other region gets its fresh start via
                # the bank's pending-zero on first touch, then accumulates.
                bank_started = {0: False, 1: False}
                for j in range(nblk):
                    if j + 1 < nblk and (j + 1) not in phats:
                        phats[j + 1] = qk_unit(i, hp, j + 1)
                    mark(f"pv{i}.{hp}.{j}")
                    dvr, o0, reg = reg_of(j)
                    lives = range(o0, 4) if causal else range(4)
                    for o in lives:
                        stag[o].append(j)
                        if j >= o:
                            for jj in stag[o]:
                                c0 = 128 * (o - shift_of(jj))
                                stop_j = (4 * i + o) if causal else (NT - 1)
                                for sl2 in range(2):
                                    bk = 0 if o < 2 else 1
                                    st = not bank_started[bk]
                                    bank_started[bk] = True
                                    nc.tensor.matmul(
                                        pv_reg(sl2, o),
                                        phats[jj][:, sl2, c0 : c0 + 128],
                                        va_t[:, jj, 2 * hp + sl2, :],
                                        start=st,
                                        stop=(jj == stop_j),
                                        skip_group_check=True,
                                    )
                            stag[o].clear()
                    if causal and dvr:
                        diag_done(o0)
                    fire(j)
                    if j >= 3:
                        for jj in [k for k in phats if k <= j]:
                            del phats[jj]
                if not causal:
                    for o in range(4):
                        diag_done(o)

            # ---------------- schedule ----------------
            if causal:
                # prelude: quarter-0 q-pair0 + k-pair0 projections + all v
                # (v first: w_v arrives before the trig tables)
                r0 = proj_group(0, 0)
                r1 = proj_group(0, 1)
                v_group(0, 0)
                proj_rot(0, 0, r0)
                v_group(0, 1)
                proj_rot(0, 1, r1)
                v_group(0, 2)
                v_group(0, 3)

                def G(q, mt, box, key):
                    def f():
                        box[key] = proj_group(q, mt)
                    return f

                def R(q, mt, box, key):
                    def f():
                        proj_rot(q, mt, box[key])
                    return f

                def V(q, st):
                    return lambda: v_group(q, st)

                def OP(i, st):
                    def f():
                        outproj_tile(i, st, [ao_tiles[(i, 0)], ao_tiles[(i, 1)]])
                    return f

                def XQ(q):
                    return lambda: issue_xq(q, nc.sync)

                bx: dict = {}
                items = {
                    (0, 0): [
                        (-2, G(0, 2, bx, "a")), (0, G(0, 3, bx, "b")),
                        (1, R(0, 2, bx, "a")), (2, R(0, 3, bx, "b")),
                    ],
                    (0, 1): [
                        (-2, XQ(2)),
                        (-2, G(1, 0, bx, "c")), (0, G(1, 1, bx, "d")),
                        (1, R(1, 0, bx, "c")), (2, V(1, 0)),
                        (3, R(1, 1, bx, "d")), (3, PFQ(1, 0, 0)),
                    ],
                    (1, 0): [
                        (-2, V(1, 1)),
                        (0, G(1, 2, bx, "e")), (1, V(1, 2)),
                        (2, R(1, 2, bx, "e")),
                        (4, G(1, 3, bx, "f")), (5, V(1, 3)),
                        (5, PFQ(1, 1, 0)),
                        (6, R(1, 3, bx, "f")), (7, PFQ(1, 1, 1)),
                    ],
                    (1, 1): [
                        (-2, XQ(3)),
                        (-2, G(2, 0, bx, "g")), (0, OP(0, 0)),
                        (2, R(2, 0, bx, "g")), (3, OP(0, 1)),
                        (4, G(2, 1, bx, "h")), (4, PFQ(2, 0, 0)),
                        (5, V(2, 0)), (5, PFQ(2, 0, 1)),
                        (6, R(2, 1, bx, "h")), (7, PFQ(2, 0, 2)),
                    ],
                    (2, 0): [
                        (-2, OP(0, 2)), (0, G(3, 0, bx, "m")),
                        (1, OP(0, 3)),
                        (2, G(2, 2, bx, "k")), (3, V(2, 1)),
                        (4, R(2, 2, bx, "k")), (5, R(3, 0, bx, "m")),
                        (6, G(2, 3, bx, "l")), (7, PFQ(2, 1, 0)),
                        (8, R(2, 3, bx, "l")), (8, V(2, 2)),
                        (9, PFQ(2, 1, 1)), (10, V(2, 3)),
                        (11, PFQ(2, 1, 2)),
                    ],
                    (2, 1): [
                        (-2, PFQ(3, 0, 0)), (0, PFQ(3, 0, 1)),
                        (2, PFQ(3, 0, 2)),
                        (3, G(3, 2, bx, "n")), (4, PFQ(3, 0, 3)),
                        (5, R(3, 2, bx, "n")), (6, PFQ(3, 0, 4)),
                        (7, V(3, 0)), (8, PFQ(3, 0, 5)), (9, V(3, 1)),
                        (10, PFQ(3, 0, 6)), (11, PFQ(3, 0, 7)),
                    ],
                    (3, 0): [
                        (-2, G(3, 1, bx, "o")), (0, R(3, 1, bx, "o")),
                        (1, OP(1, 0)), (2, V(3, 2)),
                        (4, OP(1, 1)), (6, V(3, 3)),
                        (8, OP(1, 2)), (10, OP(1, 3)),
                        (12, OP(2, 0)),
                    ],
                    (3, 1): [
                        (-2, OP(2, 2)), (2, G(3, 3, bx, "p")),
                        (4, OP(2, 3)), (6, R(3, 3, bx, "p")),
                        (8, OP(2, 1)),
                        (10, lambda: op_arm(3, 0)),
                        (12, lambda: op_fin(3, 0)),
                        (12, lambda: op_arm(3, 1)),
                        (14, lambda: op_fin(3, 1)),
                        (14, lambda: op_arm(3, 2)),
                        (15, lambda: op_fin(3, 2)),
                    ],
                }

                for i in range(NI):
                    make_attention(i, 0, items[(i, 0)])
                    make_attention(i, 1, items[(i, 1)], tail=(i == NI - 1))
                # tail: final outproj tile right after chunk 3 hp1's last
                # slice; single DMA -- the two ostage halves finish ~0.2us
                # apart, less than the extra serialized HWDGE config a
                # split-DMA pair would cost
                outproj_tile(
                    3, 3, [ao_tiles[(3, 0)], ao_tiles[(3, 1)]]
                )
                if DEBUG_TAPS:
                    for mt in range(4):
                        nc.sync.dma_start(d_qkT[mt], qkT[mt][:])
                    for i in range(NI):
                        for hp in range(2):
                            nc.sync.dma_start(d_ao[i, hp], ao_tiles[(i, hp)][:])
            else:
                # non-causal: project everything first, then attention chunks
                # with outproj(i-1) interleaved.
                issue_xq(2, nc.sync)
                for q in range(NI):
                    if q == 2:
                        issue_xq(3, nc.sync)
                    rr = [proj_group(q, mt) for mt in range(4)]
                    for mt in range(4):
                        proj_rot(q, mt, rr[mt])
                    for st in range(4):
                        v_group(q, st)
                for i in range(NI):
                    its = []
                    if i > 0:
                        ao_pair = [ao_tiles[(i - 1, 0)], ao_tiles[(i - 1, 1)]]
                        for st in range(4):
                            its.append(
                                (2 * st, (lambda s, p: lambda: outproj_tile(i - 1, s, p))(st, ao_pair))
                            )
                    make_attention(i, 0, its)
                    make_attention(i, 1, [])
                ao_pair = [ao_tiles[(NI - 1, 0)], ao_tiles[(NI - 1, 1)]]
                for st in range(4):
                    outproj_tile(NI - 1, st, ao_pair)

    nc.compile()
    return nc


# --------------------------------------------------------------------------
# host-side: constants, sharding, assembly
# --------------------------------------------------------------------------

def _rope_tables():
    inv_freq = 1.0 / (ROPE_BASE ** (np.arange(0, HD, 2, dtype=np.float32) / HD))
    positions = np.arange(S, dtype=np.float32)
    freqs = np.outer(positions, inv_freq).astype(np.float32)     # [S, 32]
    emb = np.concatenate((freqs, freqs), axis=-1)                # [S, 64]
    cosT = np.cos(emb).T.astype(np.float32)                      # [64, S]
    sinT = np.sin(emb).T.astype(np.float32)
    cos2 = np.vstack([cosT, cosT]).astype(ml_dtypes.bfloat16)    # [128, S]
    sin2 = np.vstack([sinT, sinT]).astype(ml_dtypes.bfloat16)
    return cos2, sin2


def _rot_lhsT():
    # rotate_half (interleaved): rot[2i] = -x[2i+1], rot[2i+1] = x[2i]
    P = np.zeros((128, 128), np.float32)
    for base in (0, 64):
        for i2 in range(HD // 2):
            P[base + 2 * i2, base + 2 * i2 + 1] = -1.0
            P[base + 2 * i2 + 1, base + 2 * i2] = 1.0
    return np.ascontiguousarray(P.T).astype(ml_dtypes.bfloat16)


_CACHE: dict = {}


def _get_nc(causal: bool):
    key = ("nc", causal)
    if key not in _CACHE:
        _CACHE[key] = build_nc(causal)
    return _CACHE[key]


def _classify_mask(mask: np.ndarray) -> str:
    m = np.asarray(mask).reshape(S, S)
    if np.array_equal(m != 0, np.tril(np.ones((S, S), bool))):
        return "causal"
    if np.all(m != 0):
        return "full"
    return "other"


def make_in_maps(x, w_qkv, w_out):
    """Build the 8 per-core input dicts."""
    cos2, sin2 = _rope_tables()
    rotP = _rot_lhsT()
    m01 = (np.arange(128)[:, None] <= np.arange(128)[None, :]).astype(np.float32)
    mask01 = np.ascontiguousarray(
        np.stack([m01, m01], axis=1).reshape(128, 256)
    ).astype(ml_dtypes.bfloat16)

    w3 = np.asarray(w_qkv).reshape(D, 3, H, HD)   # [D, {q,k,v}, H, hd]
    wo = np.asarray(w_out)                        # [D, D]; rows indexed [h, hd]
    xT = [
        np.ascontiguousarray(np.asarray(x)[b].T).astype(ml_dtypes.bfloat16)
        for b in range(B)
    ]  # [D, S]

    in_maps = []
    for c in range(8):
        b, hg = divmod(c, 4)
        hs = [4 * hg + i for i in range(HPC)]
        # w_qk cols: [q_h0, q_h1, k_h0, k_h1, q_h2, q_h3, k_h2, k_h3]
        wqk_cols = []
        for pair in range(2):
            for t in range(2):  # 0 = q, 1 = k
                for hh in (hs[2 * pair], hs[2 * pair + 1]):
                    wqk_cols.append(w3[:, t, hh, :])
        w_qk_c = np.ascontiguousarray(np.concatenate(wqk_cols, axis=1)).astype(
            ml_dtypes.bfloat16
        )  # [D, 512]
        w_v_c = np.ascontiguousarray(
            np.concatenate([w3[:, 2, hh, :] for hh in hs], axis=1)
        ).astype(ml_dtypes.bfloat16)  # [D, 256]
        w_out_c = np.ascontiguousarray(
            np.concatenate([wo[HD * hh : HD * hh + HD, :] for hh in hs], axis=0)
        ).astype(ml_dtypes.bfloat16)  # [256, D]
        in_maps.append(
            {
                "xT": xT[b],
                "w_qk": w_qk_c,
                "w_v": w_v_c,
                "w_out": w_out_c,
                "cos2": cos2,
                "sin2": sin2,
                "rotP": rotP,
                "ident": np.eye(128, dtype=np.float32).astype(ml_dtypes.bfloat16),
                "mask01": mask01,
            }
        )
    return in_maps


def _reference_numpy(x, mask, w_qkv, w_out):
    """Exact fallback for non-causal, non-full masks (slow, host-side)."""
    x = np.asarray(x, np.float32)
    qkv = (x @ w_qkv).reshape(B, S, 3, H, HD)
    qkv = np.transpose(qkv, (2, 0, 3, 1, 4))
    q, k, v = qkv[0], qkv[1], qkv[2]
    inv_freq = 1.0 / (ROPE_BASE ** (np.arange(0, HD, 2, dtype=np.float32) / HD))
    freqs = np.outer(np.arange(S, dtype=np.float32), inv_freq)
    emb = np.concatenate((freqs, freqs), axis=-1)
    cos = np.cos(emb)[None, None]
    sin = np.sin(emb)[None, None]

    def rot(t):
        t1 = t[..., ::2]
        t2 = t[..., 1::2]
        return np.stack((-t2, t1), axis=-1).reshape(t.shape)

    q = q * cos + rot(q) * sin
    k = k * cos + rot(k) * sin
    attn = np.einsum("bhsd,bhtd->bhst", q, k) / math.sqrt(HD)
    m = np.asarray(mask).reshape(1, 1, S, S)
    attn = np.where(m == 0, -np.inf, attn)
    attn = attn - attn.max(-1, keepdims=True)
    np.exp(attn, out=attn)
    attn /= attn.sum(-1, keepdims=True)
    out = np.einsum("bhst,bhtd->bhsd", attn, v)
    out = np.transpose(out, (0, 2, 1, 3)).reshape(B, S, D)
    return (out @ w_out).astype(np.float32)


class Runner:
    """Cached jitted SPMD runner (mirrors bass2jax.run_bass_via_pjrt)."""

    def __init__(self, nc, n_cores: int = 8):
        import jax
        import concourse.mybir as _mybir
        from concourse import bass2jax
        from jax.experimental.shard_map import shard_map
        from jax.sharding import Mesh, PartitionSpec

        bass2jax.install_neuronx_cc_hook()
        self.jax = jax
        self.n_cores = n_cores
        self._nc = nc
        in_names, out_names, out_avals, zero_outs = [], [], [], []
        for alloc in nc.m.functions[0].allocations:
            if not isinstance(alloc, _mybir.MemoryLocationSet):
                continue
            name = alloc.memorylocations[0].name
            if alloc.kind == "ExternalInput":
                in_names.append(name)
            elif alloc.kind == "ExternalOutput":
                out_names.append(name)
                shape = tuple(alloc.tensor_shape)
                dtype = _mybir.dt.np(alloc.dtype)
                out_avals.append(jax.core.ShapedArray(shape, dtype))
                zero_outs.append(np.zeros(shape, dtype))
        self.in_names = list(in_names)
        self.out_names = out_names
        self.out_avals = out_avals
        self.zero_outs = zero_outs
        all_names = in_names + out_names

        def _body(*args):
            outs = bass2jax._bass_exec_p.bind(
                *args,
                out_avals=tuple(out_avals),
                in_names=tuple(all_names),
                out_names=tuple(out_names),
                lowering_input_output_aliases=(),
                sim_require_finite=True,
                sim_require_nnan=True,
                nc=nc,
            )
            return tuple(outs)

        devices = jax.devices()[:n_cores]
        self.mesh = Mesh(np.asarray(devices), ("core",))
        n_args = len(all_names)
        self.sharded = jax.jit(
            shard_map(
                _body,
                mesh=self.mesh,
                in_specs=(PartitionSpec("core"),) * n_args,
                out_specs=(PartitionSpec("core"),) * len(out_names),
                check_rep=False,
            )
        )

    def concat_inputs(self, in_maps):
        cols = []
        for name in self.in_names:
            if name == "partition_id":
                cols.append(
                    np.arange(self.n_cores, dtype=np.uint32).reshape(self.n_cores, 1)
                )
            else:
                cols.append(
                    np.concatenate([np.asarray(m[name]) for m in in_maps], axis=0)
                )
        return cols

    def device_put(self, concat_in):
        from jax.sharding import NamedSharding, PartitionSpec

        sh = NamedSharding(self.mesh, PartitionSpec("core"))
        args = concat_in + [
            np.zeros((self.n_cores * z.shape[0], *z.shape[1:]), z.dtype)
            for z in self.zero_outs
        ]
        return [self.jax.device_put(a, sh) for a in args]

    def run_dev(self, dev_args):
        return self.sharded(*dev_args)

    def make_bench(self, n_reps: int):
        import jax
        from concourse import bass2jax
        from jax.experimental.shard_map import shard_map
        from jax.sharding import Mesh, PartitionSpec

        nc = self._nc
        out_avals = self.out_avals
        all_names = self.in_names + self.out_names
        out_names = self.out_names

        def _body(*args):
            outs = None
            for _ in range(n_reps):
                outs = bass2jax._bass_exec_p.bind(
                    *args,
                    out_avals=tuple(out_avals),
                    in_names=tuple(all_names),
                    out_names=tuple(out_names),
                    lowering_input_output_aliases=(),
                    sim_require_finite=True,
                    sim_require_nnan=True,
                    nc=nc,
                )
            return tuple(outs)

        n_args = len(all_names)
        return jax.jit(
            shard_map(
                _body,
                mesh=self.mesh,
                in_specs=(PartitionSpec("core"),) * n_args,
                out_specs=(PartitionSpec("core"),) * len(out_names),
                check_rep=False,
            )
        )

    def run(self, in_maps):
        dev_args = self.device_put(self.concat_inputs(in_maps))
        out_arrs = self.sharded(*dev_args)
        outs = []
        for c in range(self.n_cores):
            outs.append(
                {
                    name: np.asarray(out_arrs[i]).reshape(
                        self.n_cores, *self.out_avals[i].shape
                    )[c]
                    for i, name in enumerate(self.out_names)
                }
            )
        return outs


def _get_runner(causal: bool) -> Runner:
    key = ("runner", causal)
    if key not in _CACHE:
        _CACHE[key] = Runner(_get_nc(causal))
    return _CACHE[key]


def run_spmd(in_maps, causal: bool = True, **kw):
    nc = _get_nc(causal)
    return run_bass_kernel_spmd(nc, in_maps, core_ids=list(range(8)), **kw)


def kernel(x, mask, w_qkv, w_out):
    kind = _classify_mask(mask)
    if kind != "causal":
        # exact host fallback for non-causal masks (the v4 schedule is
        # specialized for the causal diagonal structure)
        return _reference_numpy(x, mask, w_qkv, w_out)
    in_maps = make_in_maps(x, w_qkv, w_out)
    res = run_spmd(in_maps, causal=True)
    out = np.zeros((B, S, D), np.float32)
    for c in range(8):
        out[c // 4] += np.asarray(res.results[c]["outp"]).astype(np.float32)
    return out


if __name__ == "__main__":
    rng = np.random.default_rng(0)
    x = rng.standard_normal((B, S, D)).astype(np.float32)
    mask = np.tril(np.ones((S, S), np.int32)).reshape(1, 1, S, S)
    w_qkv = (rng.standard_normal((D, 3 * D)) * 0.02).astype(np.float32)
    w_out = (rng.standard_normal((D, D)) * 0.02).astype(np.float32)
    got = kernel(x, mask, w_qkv, w_out)
    print("kernel ran, out shape", got.shape)
